# revision 39
# baseline (speedup 1.0000x reference)
"""Trainium2 Bass kernel: single-layer transformer encoder block.

reference:  LayerNorm -> fused QKV proj -> full softmax attention -> FC+LeakyReLU
inputs:     x [8, 2048, 512] f32 (+ LN gamma/beta, W_qkv [512,1536], W_fc [512,512], b_fc)

Sharding: pure data-parallel over batch -- each of the 8 NeuronCores gets one
batch element [S=2048, D=512]; weights are replicated, no collectives. Host
prep folds gamma/beta into W_qkv/biases and pre-casts/tiles weights to bf16.

Per-core pipeline (matmuls bf16 with f32 PSUM accumulation; ~202us HW):

  phase A  x streams in bursts on the sync DMA ring in strict consumption
           order (deterministic head latency); per 128-row tile:
           bn_stats/bn_aggr, rstd = ACT Sqrt + fast DVE reciprocal,
           xn = (x-mean)*rstd in one DVE op (single bf16 rounding),
           transpose to feature-major xnT via regular N=128 matmuls against
           the identity, then the V row-tile and (per 4 tiles) kT/qT chunks.
           Ordering edges (add_dep_helper) keep the DMA-paced bn_stats from
           head-of-line blocking the normalize chain in the in-order streams.
  phase C  per 512-query chunk: S^T = kT^T qT into paired PSUM banks (one
           [128,1024] exp per pair -> E bf16; no max-subtraction, logits are
           O(1)); softmax denominators accumulate on DVE and cost one f32r
           ones-matmul per chunk; O^T[dt] = V^T E per d-tile with 1/Z applied
           on DVE; FC back to seq-major, bias via broadcast-add, LeakyReLU as
           max(y, slope*y) on DVE; chunk-0 scores overlap phase A blocks.
"""

import numpy as np
import ml_dtypes

import concourse.bass as bass
import concourse.mybir as mybir
import concourse.tile as tile
from concourse import bacc
from concourse.bass_utils import run_bass_kernel_spmd
from concourse.masks import make_identity
from concourse.tile_rust import add_dep_helper

F32 = mybir.dt.float32
BF16 = mybir.dt.bfloat16
FP8 = mybir.dt.float8e4
F32R = mybir.dt.float32r
AF = mybir.ActivationFunctionType
OP = mybir.AluOpType

D = 512
E3 = 3 * D
ND = D // 128  # 4 feature tiles
LN_EPS = 1e-5
SLOPE = 0.01
N_CORES = 8
N_WARM = 10  # dummy PE-warmup matmuls burned during the DMA head
FILLERS = {0: 12, 1: 12, 2: 10, 3: 8, 4: 5, 5: 4}  # PE fillers after V group


def build_nc(S=2048, has_bv=False):
    NT = S // 128   # key/seq tiles
    NSC = S // 512  # query chunks
    SM_SCALE = float(D) ** -0.5

    nc = bacc.Bacc("TRN2", target_bir_lowering=False, debug=False)
    x_d = nc.dram_tensor("x", [S, D], F32, kind="ExternalInput")
    wqkv_d = nc.dram_tensor("wqkv", [128, ND, E3], BF16, kind="ExternalInput")
    wfc_d = nc.dram_tensor("wfc", [128, ND, D], BF16, kind="ExternalInput")
    bqkv_d = nc.dram_tensor("bqkv", [128, 3 * ND], F32, kind="ExternalInput")
    bfc_d = nc.dram_tensor("bfc", [1, D], BF16, kind="ExternalInput")
    out_d = nc.dram_tensor("out", [S, D], F32, kind="ExternalOutput")

    with tile.TileContext(nc) as tc:
        with (
            tc.tile_pool(name="consts", bufs=1) as consts,
            tc.tile_pool(name="persist", bufs=1) as persist,
            tc.tile_pool(name="ln", bufs=6) as lnp,
            tc.tile_pool(name="eb", bufs=2) as ebp,
            tc.tile_pool(name="zb", bufs=2) as zbp,
            tc.tile_pool(name="esb", bufs=2) as esb,
            tc.tile_pool(name="yb", bufs=3) as ybp,
            tc.tile_pool(name="psA", bufs=2, space=bass.MemorySpace.PSUM) as psA,
            tc.tile_pool(name="psO", bufs=2, space=bass.MemorySpace.PSUM) as psO,
            tc.tile_pool(name="psFC", bufs=2, space=bass.MemorySpace.PSUM) as psFC,
        ):
            # ---- tensors (allocated up front so the head DMAs can issue
            # before any const-building work lands on the queue engines) ----
            wqkv_sb = consts.tile([128, ND, E3], BF16)
            wfc_sb = consts.tile([128, ND, D], BF16)
            bqkv_sb = consts.tile([128, 3 * ND], F32)
            bfc_sb = consts.tile([128, D], BF16)
            x_r = x_d.rearrange("(t p) d -> p t d", p=128)
            x_tiles = persist.tile([128, NT, D], F32, name="x_tiles")

            # ---- persistent activations ----
            xnT = persist.tile([128, ND, S], BF16)   # xn^T: [d_in_tile, d_tile, s]
            qT = persist.tile([128, ND, S], BF16)    # q^T: [e_in_tile, e_tile, s]
            kT = persist.tile([128, ND, S], BF16)
            vv = persist.tile([128, NT, D], BF16)    # V: [t_in_tile, t_tile, d]

            # ---- head DMAs: x bursts first on the sync ring, in strict
            # consumption order (1,1,2,[Wk],4,[Wq],8 tiles); the first burst
            # is a single tile so the LN chain starts ~1.2us earlier. The
            # gpsimd ring carries W_v (gates the first V matmuls) + bqkv.
            def _xburst(eng, lo, hi):
                lo = min(lo, NT)
                hi = min(hi, NT)
                if lo < hi:
                    eng.dma_start(out=x_tiles[:, lo:hi, :],
                                  in_=x_r[:, lo:hi, :])
            def _wpiece(eng, w):
                eng.dma_start(out=wqkv_sb[:, :, w * D:(w + 1) * D],
                              in_=wqkv_d[:, :, w * D:(w + 1) * D])
            # sync ring carries ONLY x (full HBM share for the PE-starved
            # head), issued PER TILE so each tile gets its own completion
            # semaphore -- coarse bursts made bn_stats(i) wait for a whole
            # 1-2MB burst even though tile i had landed long before. All
            # weights ride the gpsimd ring, in need order.
            _xburst(nc.sync, 0, 1)
            _wpiece(nc.gpsimd, 2)   # W_v
            nc.gpsimd.dma_start(out=bqkv_sb, in_=bqkv_d[:])
            for t in range(1, NT):
                _xburst(nc.sync, t, t + 1)
                if t == 2:
                    _wpiece(nc.gpsimd, 1)   # W_k
                if t == 6:
                    _wpiece(nc.gpsimd, 0)   # W_q

            # ---- PE clock warmup ----
            # The PE HAM clock-gate only reaches K=8/8 after ~3.4us of
            # sustained matmul activity; without this the whole LN/V/kq head
            # runs at 1.2 GHz until ~26us in. Burn the DMA-wait head (PE is
            # otherwise idle until ~11us) with dummy matmuls so the real
            # phase-A matmuls start warm. Output goes to the (phase-C-only)
            # psO pool and is never read.
            warm_sb = consts.tile([128, 512], BF16)
            nc.vector.memset(warm_sb, 0.5)
            wps = psO.tile([128, 512], F32, tag="o", name="wps")
            for _ in range(N_WARM):
                nc.tensor.matmul(wps, warm_sb[:, :128], warm_sb,
                                 start=True, stop=True, skip_group_check=True)

            # ---- constants ----
            zero_sb = consts.tile([128, 1], F32)
            nc.vector.memset(zero_sb, 0.0)
            # The kernel's only ACT functions are Exp and Identity -- both
            # live in the same table set (exp_and_others), so exactly one
            # ACT_TABLE_LOAD is ever needed. Trigger it here in the DMA-wait
            # head with a dummy exp; Sqrt (a different set that would thrash
            # against Exp) is avoided entirely via the DVE rsqrt below.
            tscr = consts.tile([128, 1], F32)
            nc.scalar.activation(out=tscr, in_=zero_sb, func=AF.Exp,
                                 bias=zero_sb)
            ident = consts.tile([128, 128], BF16)
            make_identity(nc, ident)
            ones_f = consts.tile([128, 128], F32)
            nc.vector.memset(ones_f, 1.0)
            ones_r = consts.tile([128, 128], F32R)
            nc.vector.tensor_copy(out=ones_r, in_=ones_f)

            nc.gpsimd.dma_start(out=wfc_sb, in_=wfc_d[:])
            bfc_bcast = bass.AP(
                tensor=bfc_d.ap().tensor, offset=0,
                ap=[[0, 128]] + bfc_d.ap().ap[1:])
            nc.gpsimd.dma_start(out=bfc_sb, in_=bfc_bcast)

            def emit_score_pairs(sc, E, esum, tp_lo, tp_hi, eng=None):
                # scores + exp; the softmax denominators accumulate on `eng`
                # so the PE only pays one f32r ones-matmul per chunk for the
                # cross-partition sum. Phase-A prechunks accumulate on the
                # otherwise-idle GPSIMD (its ~1.2us/op serial chain finishes
                # long before phase C needs it); phase-C chunks use the DVE
                # (~0.55us/op), whose chain tracks the exps closely enough
                # that the Z matmul never waits.
                if eng is None:
                    eng = nc.vector
                for tp in range(tp_lo, tp_hi):
                    ps = psA.tile([128, 2, 512], F32, tag="mm", name="pss")
                    for half in range(2):
                        tt = 2 * tp + half
                        for et in range(ND):
                            nc.tensor.matmul(
                                ps[:, half, :],
                                kT[:, et, tt * 128:(tt + 1) * 128],
                                qT[:, et, sc * 512:(sc + 1) * 512],
                                start=(et == 0), stop=(et == ND - 1),
                            )
                    nc.scalar.activation(
                        out=E[:, 2 * tp:2 * tp + 2, :], in_=ps, func=AF.Exp,
                        bias=zero_sb, scale=SM_SCALE,
                    )
                    if tp == 0:
                        eng.tensor_copy(out=esum, in_=E[:, 0, :])
                        eng.tensor_add(out=esum, in0=esum, in1=E[:, 1, :])
                    else:
                        for half in range(2):
                            eng.tensor_add(out=esum, in0=esum,
                                           in1=E[:, 2 * tp + half, :])

            # ---- phase A, per tile: LN chain, transpose, V; per group of
            # 4 tiles: the k and q projections plus score pairs for the
            # first PRE chunks as their kT inputs land. Spreading the
            # chunk-0/1 scores across phase A keeps the PE saturated while
            # x streams in, and gets those exps off the phase-C critical
            # path.
            xn_insts = []
            prechunks = {}  # sc -> [E, esum, done_pairs]
            PRE = min(2, NSC)
            for it in range(NT):
                stat = lnp.tile([128, 6], F32, tag="stat")
                bn_inst = nc.vector.bn_stats(out=stat, in_=x_tiles[:, it, :])
                if it >= 2:
                    # keep the DVE stream interleaved: without this edge the
                    # scheduler front-loads all (DMA-paced) bn_stats and the
                    # normalize chain head-of-line blocks behind them
                    add_dep_helper(bn_inst.ins, xn_insts[it - 2].ins,
                                   sync=False, reason="interleave LN chain")
                mv = lnp.tile([128, 2], F32, tag="mv")
                nc.vector.bn_aggr(out=mv, in_=stat)
                veps = lnp.tile([128, 1], F32, tag="veps")
                rstd = lnp.tile([128, 1], F32, tag="rstd")
                nra = lnp.tile([128, 1], F32, tag="nra")
                xn = lnp.tile([128, D], BF16, tag="xn")
                # rstd = 1/sqrt(var+eps) entirely on DVE: Quake bit-trick
                # seed + 2 Newton steps (max rel err 4.7e-6). Avoids ACT
                # Sqrt, whose table set would thrash against the Exp set.
                # high priority: don't let later (DMA-paced) bn_stats get
                # ahead of the normalize chain in the in-order DVE stream
                with tc.high_priority():
                    nc.vector.tensor_scalar_add(out=veps, in0=mv[:, 1:2],
                                                scalar1=LN_EPS)
                    ru = rstd.bitcast(mybir.dt.uint32)
                    vu = veps.bitcast(mybir.dt.uint32)
                    # seed bits = 0x5f3759df - u/2, done in FLOAT arithmetic
                    # (DVE converts uint32 operands value-wise; its add/mult
                    # are fp32 ops, so integer two's-complement tricks don't
                    # work). The +-64 ULP f32 rounding is absorbed by the
                    # Newton steps.
                    nc.vector.tensor_scalar(
                        out=ru, in0=vu, scalar1=-0.5,
                        scalar2=float(0x5F3759DF),
                        op0=OP.mult, op1=OP.add)
                    for _ in range(2):
                        nc.vector.tensor_mul(out=nra, in0=rstd, in1=rstd)
                        nc.vector.tensor_mul(out=nra, in0=nra, in1=veps)
                        nc.vector.tensor_scalar(
                            out=nra, in0=nra, scalar1=-0.5, scalar2=1.5,
                            op0=OP.mult, op1=OP.add)
                        nc.vector.tensor_mul(out=rstd, in0=rstd, in1=nra)
                    xn_insts.append(nc.vector.tensor_scalar(
                        out=xn, in0=x_tiles[:, it, :], scalar1=mv[:, 0:1],
                        scalar2=rstd, op0=OP.subtract, op1=OP.mult,
                    ))
                # transpose via regular N=128 bf16 matmuls vs identity;
                # lands in the (phase-A-idle) FC psum pool
                pt = psFC.tile([128, ND, 128], F32, tag="fc", name="pt")
                for j in range(ND):
                    nc.tensor.matmul(
                        pt[:, j, :],
                        xn[:, j * 128:(j + 1) * 128],
                        ident,
                        start=True, stop=True,
                    )
                nc.scalar.activation(
                    out=xnT[:, :, it * 128:(it + 1) * 128], in_=pt,
                    func=AF.Identity, bias=zero_sb,
                )
                # V row-tile: ready as soon as this xnT tile lands
                ps = psA.tile([128, 512], F32, tag="mm", name="psv")
                for dt in range(ND):
                    nc.tensor.matmul(
                        ps,
                        xnT[:, dt, it * 128:(it + 1) * 128],
                        wqkv_sb[:, dt, 2 * D:3 * D],
                        start=(dt == 0), stop=(dt == ND - 1),
                    )
                nc.scalar.activation(out=vv[:, it, :], in_=ps,
                                     func=AF.Identity, bias=zero_sb)

                # fillers: the first tiles are DMA-paced; keep the PE array
                # busy through the holes so the HAM clock-gate doesn't
                # re-throttle to 1.2 GHz.
                if it in FILLERS:
                    for _ in range(FILLERS[it]):
                        nc.tensor.matmul(
                            wps[:, :128], warm_sb[:, :128], warm_sb[:, :128],
                            start=True, stop=True, skip_group_check=True)

                if it % 4 != 3:
                    continue
                # group projections: k, then score pairs of already-started
                # chunks (they need only kT(g) + their own qT), then q, then
                # this group's chunk joins and backfills all available pairs.
                g = it // 4
                for et in range(ND):
                    ps = psA.tile([128, 512], F32, tag="mm", name="psk")
                    for dt in range(ND):
                        nc.tensor.matmul(
                            ps,
                            wqkv_sb[:, dt, D + et * 128: D + (et + 1) * 128],
                            xnT[:, dt, g * 512:(g + 1) * 512],
                            start=(dt == 0), stop=(dt == ND - 1),
                        )
                    nc.scalar.activation(
                        out=kT[:, et, g * 512:(g + 1) * 512], in_=ps,
                        func=AF.Identity,
                        bias=bqkv_sb[:, ND + et: ND + et + 1],
                    )
                for sc, pc in prechunks.items():
                    emit_score_pairs(sc, pc[0], pc[1], 2 * g, 2 * g + 2,
                                     eng=nc.gpsimd)
                    pc[2] = 2 * g + 2
                for et in range(ND):
                    ps = psA.tile([128, 512], F32, tag="mm", name="psq")
                    for dt in range(ND):
                        nc.tensor.matmul(
                            ps,
                            wqkv_sb[:, dt, et * 128:(et + 1) * 128],
                            xnT[:, dt, g * 512:(g + 1) * 512],
                            start=(dt == 0), stop=(dt == ND - 1),
                        )
                    # q copy+bias on ACT: on the in-order DVE stream these
                    # adds head-of-line blocked the next tiles' xn, which
                    # the PE was stalled waiting for
                    nc.scalar.activation(
                        out=qT[:, et, g * 512:(g + 1) * 512], in_=ps,
                        func=AF.Identity,
                        bias=bqkv_sb[:, et:et + 1],
                    )
                if g < PRE and NSC > 1:
                    E = ebp.tile([128, NT, 512], BF16, tag="E", name=f"E{g}")
                    es = esb.tile([128, 512], F32R, tag="es", name=f"es{g}")
                    emit_score_pairs(g, E, es, 0, 2 * g + 2, eng=nc.gpsimd)
                    prechunks[g] = [E, es, 2 * g + 2]

            # ---- phase C: attention + FC, per query chunk ----
            for sc in range(NSC):
                if sc in prechunks:
                    E, esum, done = prechunks[sc]
                    emit_score_pairs(sc, E, esum, done, NT // 2,
                                     eng=nc.gpsimd)
                else:
                    E = ebp.tile([128, NT, 512], BF16, tag="E")
                    esum = esb.tile([128, 512], F32R, tag="es", name="esum")
                    emit_score_pairs(sc, E, esum, 0, NT // 2)
                zinv = zbp.tile([128, 512], F32, tag="zinv")
                oT = ebp.tile([128, ND, 512], BF16, tag="oT")
                zp = None
                last = sc == NSC - 1
                if last:
                    # last chunk: FC accumulates per-dt into four psA-borrowed
                    # banks, interleaved between PV groups, so the post-PV
                    # serial tail is one 4-MM pass instead of 16 MMs
                    fca = psA.tile([128, 2, 512], F32, tag="mm", name="fca")
                    fcb = psA.tile([128, 2, 512], F32, tag="mm", name="fcb")
                    fcs = [fca[:, 0, :], fca[:, 1, :],
                           fcb[:, 0, :], fcb[:, 1, :]]

                    def emit_fc_dt(dt):
                        for ss in range(4):
                            nc.tensor.matmul(
                                fcs[ss],
                                oT[:, dt, ss * 128:(ss + 1) * 128],
                                wfc_sb[:, dt, :],
                                start=(dt == 0), stop=(dt == ND - 1),
                            )
                for dt in range(ND):
                    op = psO.tile([128, 512], F32, tag="o", name=f"op{dt}")
                    for tt in range(NT):
                        nc.tensor.matmul(
                            op,
                            vv[:, tt, dt * 128:(dt + 1) * 128],
                            E[:, tt, :],
                            start=(tt == 0), stop=(tt == NT - 1),
                        )
                    if dt == 0:
                        # Z after the first PV pass: PV needs only E, so the
                        # PE isn't stalled waiting for the esum tail
                        zp = psFC.tile([128, 512], F32, tag="fc", name="zp")
                        nc.tensor.matmul(zp, ones_r, esum,
                                         start=True, stop=True)
                        nc.vector.reciprocal_approx_fast(out=zinv, in_=zp)
                    if last and dt >= 1:
                        emit_fc_dt(dt - 1)
                    nc.vector.tensor_mul(out=oT[:, dt, :], in0=op, in1=zinv)
                    if has_bv:
                        nc.vector.tensor_scalar_add(
                            out=oT[:, dt, :], in0=oT[:, dt, :],
                            scalar1=bqkv_sb[:, 2 * ND + dt: 2 * ND + dt + 1],
                        )

                def emit_fc_out(ss, ps):
                    # y = ps + b_fc (broadcast rows), LeakyReLU via DVE
                    # (overlaps the next chunk's scores on the PE)
                    yb = ybp.tile([128, D], F32, tag="yb")
                    nc.vector.tensor_add(out=yb, in0=ps, in1=bfc_sb)
                    yt = ybp.tile([128, D], F32, tag="y")
                    nc.vector.scalar_tensor_tensor(
                        out=yt, in0=yb, scalar=SLOPE, in1=yb,
                        op0=OP.mult, op1=OP.max,
                    )
                    r0 = sc * 512 + ss * 128
                    if sc == NSC - 1:
                        # tail: split across two rings so the final 1MB
                        # drains in half the time after the last FC
                        nc.sync.dma_start(out=out_d[r0:r0 + 128, :D // 2],
                                          in_=yt[:, :D // 2])
                        nc.gpsimd.dma_start(out=out_d[r0:r0 + 128, D // 2:],
                                            in_=yt[:, D // 2:])
                    else:
                        nc.sync.dma_start(out=out_d[r0:r0 + 128, :], in_=yt)

                if last:
                    emit_fc_dt(ND - 1)
                    for ss in range(4):
                        emit_fc_out(ss, fcs[ss])
                else:
                    for ss in range(4):
                        ps = psFC.tile([128, 512], F32, tag="fc")
                        for dt in range(ND):
                            nc.tensor.matmul(
                                ps,
                                oT[:, dt, ss * 128:(ss + 1) * 128],
                                wfc_sb[:, dt, :],
                                start=(dt == 0), stop=(dt == ND - 1),
                            )
                        emit_fc_out(ss, ps)

    nc.compile()
    return nc


_NC_CACHE = {}


def _get_nc(S, has_bv):
    key = (S, has_bv)
    if key not in _NC_CACHE:
        _NC_CACHE[key] = build_nc(S, has_bv)
    return _NC_CACHE[key]


def prep_inputs(x, ln_gamma, ln_beta, W_qkv, W_fc, b_fc):
    bf = ml_dtypes.bfloat16
    W_qkv = np.asarray(W_qkv, dtype=np.float32)
    Wq = W_qkv * np.asarray(ln_gamma, dtype=np.float32)[:, None]
    wqkv_t = np.ascontiguousarray(
        Wq.reshape(ND, 128, E3).transpose(1, 0, 2)).astype(bf)
    wfc_t = np.ascontiguousarray(
        np.asarray(W_fc, dtype=np.float32).reshape(ND, 128, D).transpose(1, 0, 2)
    ).astype(bf)
    bqkv = np.asarray(ln_beta, dtype=np.float32) @ W_qkv  # [1536]
    bqkv_t = np.ascontiguousarray(bqkv.reshape(3 * ND, 128).T)
    bfc_t = np.asarray(b_fc, dtype=np.float32).reshape(1, D).astype(bf)
    has_bv = bool(np.any(bqkv[2 * D:]))
    return wqkv_t, wfc_t, bqkv_t, bfc_t, has_bv


def run(x, ln_gamma, ln_beta, W_qkv, W_fc, b_fc, trace=False):
    x = np.asarray(x, dtype=np.float32)
    B, S, Din = x.shape
    assert B == N_CORES and Din == D and S % 512 == 0, (B, S, Din)
    wqkv_t, wfc_t, bqkv_t, bfc_t, has_bv = prep_inputs(
        x, ln_gamma, ln_beta, W_qkv, W_fc, b_fc)
    nc = _get_nc(S, has_bv)
    in_maps = [
        {
            "x": np.ascontiguousarray(x[b]),
            "wqkv": wqkv_t,
            "wfc": wfc_t,
            "bqkv": bqkv_t,
            "bfc": bfc_t,
        }
        for b in range(B)
    ]
    res = run_bass_kernel_spmd(nc, in_maps, core_ids=list(range(B)), trace=trace)
    out = np.stack([res.results[b]["out"] for b in range(B)]).astype(np.float32)
    return out, res


def kernel(x, ln_gamma, ln_beta, W_qkv, W_fc, b_fc):
    out, _ = run(x, ln_gamma, ln_beta, W_qkv, W_fc, b_fc)
    return out

